# revision 6
# baseline (speedup 1.0000x reference)
"""Causal self-attention with RoPE on 8 Trainium2 NeuronCores.

Sharding: tensor-parallel over heads (2 heads/core) for QKV+attention,
then an AllToAll redistributes y^T from head-shards to token-shards and
each core projects its 512-token slice with the full W_proj.

All matmuls run in float32r (TF32) with fp32 PSUM accumulation.
"""

import math

import numpy as np

import concourse.bass as bass
import concourse.mybir as mybir
import concourse.tile as tile
from concourse import bacc
from concourse.bass_utils import run_bass_kernel_spmd
from concourse.masks import make_identity

# Problem shape (hardcoded per contest rules).
B, T, D = 2, 2048, 1024
H, DH = 16, 64
ROPE_BASE = 10000.0
N_CORES = 8
HEADS_PER_CORE = H // N_CORES          # 2
P = 128
N_STRIPS = T // 512                    # 4 strips of 512 tokens per batch
TOK = B * T                            # 4096 flat tokens
TOK_PER_CORE = TOK // N_CORES          # 512

FP32 = mybir.dt.float32
FP32R = mybir.dt.float32r
AF = mybir.ActivationFunctionType
ALU = mybir.AluOpType


def _build_program():
    nc = bacc.Bacc(None, target_bir_lowering=False, debug=False)

    x_d = nc.dram_tensor("x", [TOK, D], FP32, kind="ExternalInput")
    wq_d = nc.dram_tensor("wq", [D, P], FP32, kind="ExternalInput")
    wk_d = nc.dram_tensor("wk", [D, P], FP32, kind="ExternalInput")
    wv_d = nc.dram_tensor("wv", [D, P], FP32, kind="ExternalInput")
    wp_d = nc.dram_tensor("wp", [D, D], FP32, kind="ExternalInput")
    cos_d = nc.dram_tensor("cos", [P, T], FP32, kind="ExternalInput")
    sin_d = nc.dram_tensor("sin", [P, T], FP32, kind="ExternalInput")
    ones_d = nc.dram_tensor("ones", [P, 1], FP32, kind="ExternalInput")
    out_d = nc.dram_tensor("out", [TOK_PER_CORE, D], FP32, kind="ExternalOutput")

    DC = D // P  # 8 contraction chunks

    with tile.TileContext(nc) as tc:
        with (
            tc.tile_pool(name="const", bufs=1) as cpool,
            tc.tile_pool(name="ptr", bufs=2, space="PSUM") as ptr,
            tc.tile_pool(name="pmm", bufs=2, space="PSUM") as pmm,
            tc.tile_pool(name="ps", bufs=2, space="PSUM") as ps,
            tc.tile_pool(name="py", bufs=2, space="PSUM") as py,
            tc.tile_pool(name="dram", bufs=1, space="DRAM") as dram,
        ):
            ident = cpool.tile([P, P], FP32)
            make_identity(nc, ident[:])

            w_sb = {}
            for name, wd in (("q", wq_d), ("k", wk_d), ("v", wv_d)):
                w_sb[name] = cpool.tile([P, DC, P], FP32R, tag=f"w{name}", name=f"w{name}")
                nc.sync.dma_start(
                    w_sb[name][:],
                    wd[:].rearrange("(o p) j -> p o j", p=P).bitcast(FP32R),
                )
            cos_sb = cpool.tile([P, T], FP32)
            sin_sb = cpool.tile([P, T], FP32)
            nc.sync.dma_start(cos_sb[:], cos_d[:])
            nc.sync.dma_start(sin_sb[:], sin_d[:])
            ones_sb = cpool.tile([P, 1], FP32)
            nc.sync.dma_start(ones_sb[:], ones_d[:])

            # y^T accumulators for the whole run (head A / head B rows kept in
            # separate tiles so every DVE write stays lane-aligned).
            y2t = [cpool.tile([64, TOK], FP32R, tag=f"y2t{h}", name=f"y2t{h}") for h in range(2)]

            a2a_in = dram.tile([TOK // 512 * P, 512], FP32)
            a2a_out = dram.tile([TOK // 512 * P, 512], FP32)

            with (
                tc.tile_pool(name="work", bufs=2) as work,
                tc.tile_pool(name="ptp", bufs=3) as ptp,
            ):
                for b in range(B):
                    # ---- QKV projection (+ on-the-fly x transpose) ----
                    qt_f = work.tile([P, T], FP32R, tag="qt_f")
                    kt_f = work.tile([P, T], FP32R, tag="kt_f")
                    va = work.tile([P, T // P, 65], FP32R, tag="va")
                    vb = work.tile([P, T // P, 65], FP32R, tag="vb")
                    nc.vector.tensor_copy(
                        va[:, :, 64], ones_sb[:, 0:1].to_broadcast((P, T // P))
                    )
                    nc.vector.tensor_copy(
                        vb[:, :, 64], ones_sb[:, 0:1].to_broadcast((P, T // P))
                    )
                    for s in range(N_STRIPS):
                        xt = work.tile([P, DC, 512], FP32R, tag="xt")
                        for tt in range(4):
                            xn = work.tile([P, D], FP32, tag="xn")
                            r0 = b * T + s * 512 + tt * P
                            nc.sync.dma_start(xn[:], x_d[r0 : r0 + P, :])
                            for dc in range(DC):
                                pt_ = ptr.tile([P, P], FP32, tag="ptr")
                                nc.tensor.transpose(
                                    pt_[:], xn[:, dc * P : (dc + 1) * P], ident[:]
                                )
                                nc.vector.tensor_copy(
                                    xt[:, dc, tt * P : (tt + 1) * P], pt_[:]
                                )
                        sl = slice(s * 512, (s + 1) * 512)
                        # Q and K: project, then RoPE into qt_f/kt_f.
                        for name, dst in (("q", qt_f), ("k", kt_f)):
                            pm = pmm.tile([P, 512], FP32, tag="pmm")
                            for dc in range(DC):
                                nc.tensor.matmul(
                                    pm[:],
                                    w_sb[name][:, dc],
                                    xt[:, dc],
                                    start=(dc == 0),
                                    stop=(dc == DC - 1),
                                )
                            raw = work.tile([P, 512], FP32, tag="raw")
                            nc.vector.tensor_copy(raw[:], pm[:])
                            perm = work.tile([P, 512], FP32, tag="perm")
                            for blk in range(4):
                                p0 = blk * 32
                                src = p0 + 32 if blk % 2 == 0 else p0 - 32
                                nc.sync.dma_start(
                                    perm[p0 : p0 + 32, :], raw[src : src + 32, :]
                                )
                            # rope: dst = raw*cos + perm*sin_signed
                            nc.vector.tensor_tensor(
                                raw[:], raw[:], cos_sb[:, sl], ALU.mult
                            )
                            nc.vector.tensor_tensor(
                                perm[:], perm[:], sin_sb[:, sl], ALU.mult
                            )
                            nc.vector.tensor_tensor(
                                dst[:, sl], raw[:], perm[:], ALU.add
                            )
                        # V: project then transpose into per-head [tk, 65] tiles.
                        pm = pmm.tile([P, 512], FP32, tag="pmm")
                        for dc in range(DC):
                            nc.tensor.matmul(
                                pm[:],
                                w_sb["v"][:, dc],
                                xt[:, dc],
                                start=(dc == 0),
                                stop=(dc == DC - 1),
                            )
                        vt = work.tile([P, 512], FP32, tag="vt")
                        nc.vector.tensor_copy(vt[:], pm[:])
                        for tt in range(4):
                            ptv = ptr.tile([P, P], FP32, tag="ptr")
                            nc.tensor.transpose(
                                ptv[:], vt[:, tt * P : (tt + 1) * P], ident[:]
                            )
                            nc.vector.tensor_copy(
                                va[:, s * 4 + tt, 0:64], ptv[:, 0:64]
                            )
                            nc.vector.tensor_copy(
                                vb[:, s * 4 + tt, 0:64], ptv[:, 64:128]
                            )

                    # ---- attention ----
                    for s in range(N_STRIPS):
                        qsl = slice(s * 512, (s + 1) * 512)
                        for h in range(2):
                            ph = 64 * h
                            v_h = va if h == 0 else vb
                            pyt = py.tile([65, 512], FP32, tag="py")
                            jmax = 4 * s + 3
                            for j in range(jmax + 1):
                                pss = ps.tile([P, 512], FP32, tag="ps")
                                nc.tensor.matmul(
                                    pss[:],
                                    kt_f[ph : ph + 64, j * P : (j + 1) * P],
                                    qt_f[ph : ph + 64, qsl],
                                    start=True,
                                    stop=True,
                                )
                                pt = ptp.tile([P, 512], FP32R, tag="pt")
                                nc.scalar.activation(
                                    pt[:], pss[:], AF.Exp, scale=1.0 / math.sqrt(DH)
                                )
                                if j >= 4 * s:
                                    # keep where (512s + col) - (128j + row) >= 0
                                    nc.gpsimd.affine_select(
                                        out=pt[:],
                                        in_=pt[:],
                                        compare_op=ALU.is_ge,
                                        fill=0.0,
                                        base=512 * s - 128 * j,
                                        channel_multiplier=-1,
                                        pattern=[[1, 512]],
                                    )
                                nc.tensor.matmul(
                                    pyt[:],
                                    v_h[:, j, :],
                                    pt[:],
                                    start=(j == 0),
                                    stop=(j == jmax),
                                )
                            r65 = work.tile([65, 512], FP32, tag="r65")
                            nc.vector.reciprocal(r65[64:65, :], pyt[64:65, :])
                            r_dram = dram.tile([1, 512], FP32, tag="r_dram", name="r_dram")
                            nc.sync.dma_start(r_dram[:], r65[64:65, :])
                            rb = work.tile([64, 512], FP32, tag="rb")
                            nc.sync.dma_start(
                                rb[:], r_dram[:].to_broadcast((64, 512))
                            )
                            nc.vector.tensor_tensor(
                                y2t[h][:, b * T + s * 512 : b * T + (s + 1) * 512],
                                pyt[0:64, :],
                                rb[:],
                                ALU.mult,
                            )

            # ---- AllToAll: head-shards -> token-shards ----
            for j in range(TOK // 512):
                jsl = slice(j * 512, (j + 1) * 512)
                nc.sync.dma_start(
                    a2a_in[j * P : j * P + 64, :], y2t[0][:, jsl].bitcast(FP32)
                )
                nc.sync.dma_start(
                    a2a_in[j * P + 64 : (j + 1) * P, :], y2t[1][:, jsl].bitcast(FP32)
                )
            nc.gpsimd.collective_compute(
                "AllToAll",
                ALU.bypass,
                replica_groups=[list(range(N_CORES))],
                ins=[a2a_in.opt()],
                outs=[a2a_out.opt()],
            )

            # ---- projection of this core's 512 tokens with full W_proj ----
            with (
                tc.tile_pool(name="projc", bufs=1) as projc,
                tc.tile_pool(name="proj", bufs=2) as proj,
            ):
                wp_sb = projc.tile([P, DC, D], FP32R, tag="wp")
                nc.sync.dma_start(
                    wp_sb[:], wp_d[:].rearrange("(o p) j -> p o j", p=P).bitcast(FP32R)
                )
                yt_sb = projc.tile([P, DC, 512], FP32R, tag="yt")
                nc.sync.dma_start(
                    yt_sb[:],
                    a2a_out[:].rearrange("(o p) t -> p o t", p=P).bitcast(FP32R),
                )
                for tt in range(4):
                    for oc in range(2):
                        pm = pmm.tile([P, 512], FP32, tag="pmm")
                        for dc in range(DC):
                            nc.tensor.matmul(
                                pm[:],
                                yt_sb[:, dc, tt * P : (tt + 1) * P],
                                wp_sb[:, dc, oc * 512 : (oc + 1) * 512],
                                start=(dc == 0),
                                stop=(dc == DC - 1),
                            )
                        ob = proj.tile([P, 512], FP32, tag="ob")
                        nc.vector.tensor_copy(ob[:], pm[:])
                        nc.sync.dma_start(
                            out_d[tt * P : (tt + 1) * P, oc * 512 : (oc + 1) * 512],
                            ob[:],
                        )

    nc.compile()
    return nc


_NC_CACHE = None


def _get_program():
    global _NC_CACHE
    if _NC_CACHE is None:
        _NC_CACHE = _build_program()
    return _NC_CACHE


def _host_tables():
    inv_freq = 1.0 / (ROPE_BASE ** (np.arange(0, DH, 2, dtype=np.float32) / DH))
    t = np.arange(T, dtype=np.float32)
    freqs = np.outer(t, inv_freq).astype(np.float32)  # (T, 32)
    cos_t = np.cos(freqs).T                           # (32, T)
    sin_t = np.sin(freqs).T
    cos = np.empty((P, T), np.float32)
    sin = np.empty((P, T), np.float32)
    for blk in range(4):
        cos[blk * 32 : (blk + 1) * 32] = cos_t
        # rotate_half: row p<32 pairs with -q[p+32]; row p>=32 with +q[p-32]
        sgn = -1.0 if blk % 2 == 0 else 1.0
        sin[blk * 32 : (blk + 1) * 32] = sgn * sin_t
    return cos, sin


def make_in_maps(x, W_qkv, W_proj):
    x = np.asarray(x, np.float32).reshape(TOK, D)
    W_qkv = np.asarray(W_qkv, np.float32)
    W_proj = np.asarray(W_proj, np.float32)
    cos, sin = _host_tables()
    ones = np.ones((P, 1), np.float32)

    in_maps = []
    for c in range(N_CORES):
        j0 = c * P
        in_maps.append(
            {
                "x": x,
                "wq": np.ascontiguousarray(W_qkv[:, j0 : j0 + P]),
                "wk": np.ascontiguousarray(W_qkv[:, D + j0 : D + j0 + P]),
                "wv": np.ascontiguousarray(W_qkv[:, 2 * D + j0 : 2 * D + j0 + P]),
                "wp": W_proj,
                "cos": cos,
                "sin": sin,
                "ones": ones,
            }
        )
    return in_maps


def kernel(x, W_qkv, W_proj):
    in_maps = make_in_maps(x, W_qkv, W_proj)
    nc = _get_program()
    res = run_bass_kernel_spmd(nc, in_maps, list(range(N_CORES)))
    out = np.concatenate([res.results[c]["out"] for c in range(N_CORES)], axis=0)
    return out.reshape(B, T, D)


# revision 8
# speedup vs baseline: 96.8250x; 96.8250x over previous
"""Causal self-attention with RoPE on 8 Trainium2 NeuronCores.

Sharding: tensor-parallel over heads (2 heads/core) for QKV+attention,
then an AllToAll redistributes y^T from head-shards to token-shards and
each core projects its 512-token slice with the full W_proj.

All matmuls run in float32r (TF32) with fp32 PSUM accumulation.
"""

import math

import numpy as np

import concourse.bass as bass
import concourse.mybir as mybir
import concourse.tile as tile
from concourse import bacc
from concourse.bass_utils import run_bass_kernel_spmd
from concourse.masks import make_identity

# Problem shape (hardcoded per contest rules).
B, T, D = 2, 2048, 1024
H, DH = 16, 64
ROPE_BASE = 10000.0
N_CORES = 8
HEADS_PER_CORE = H // N_CORES          # 2
P = 128
N_STRIPS = T // 512                    # 4 strips of 512 tokens per batch
TOK = B * T                            # 4096 flat tokens
TOK_PER_CORE = TOK // N_CORES          # 512
DC = D // P                            # 8 contraction chunks

FP32 = mybir.dt.float32
FP32R = mybir.dt.float32r
AF = mybir.ActivationFunctionType
ALU = mybir.AluOpType


def _emit_body(nc, tc, d, consts):
    """One full forward pass; emitted `reps` times for slope timing."""
    cpool = consts["cpool"]
    dram = consts["dram"]
    ptr, pmm, ps, py = consts["ptr"], consts["pmm"], consts["ps"], consts["py"]
    ident, w_sb = consts["ident"], consts["w_sb"]
    cos_sb, sin_sb, ones_sb = consts["cos_sb"], consts["sin_sb"], consts["ones_sb"]

    # y^T accumulators (head A / head B rows kept in separate tiles so every
    # DVE write stays lane-aligned).
    y2t = [cpool.tile([64, TOK], FP32R, tag=f"y2t{h}", name=f"y2t{h}")
           for h in range(2)]
    a2a_in = dram.tile([TOK // 512 * P, 512], FP32, tag="a2a_in", name="a2a_in")
    a2a_out = dram.tile([TOK // 512 * P, 512], FP32, tag="a2a_out", name="a2a_out")

    with (
        tc.tile_pool(name="work", bufs=2) as work,
        tc.tile_pool(name="ptp", bufs=3) as ptp,
    ):
        for b in range(B):
            # ---- QKV projection (+ on-the-fly x transpose) ----
            qt_f = work.tile([P, T], FP32R, tag="qt_f")
            kt_f = work.tile([P, T], FP32R, tag="kt_f")
            va = work.tile([P, T // P, 65], FP32R, tag="va")
            vb = work.tile([P, T // P, 65], FP32R, tag="vb")
            nc.vector.tensor_copy(
                va[:, :, 64], ones_sb[:, 0:1].to_broadcast((P, T // P))
            )
            nc.vector.tensor_copy(
                vb[:, :, 64], ones_sb[:, 0:1].to_broadcast((P, T // P))
            )
            for s in range(N_STRIPS):
                xt = work.tile([P, DC, 512], FP32R, tag="xt")
                for tt in range(4):
                    xn = work.tile([P, D], FP32, tag="xn")
                    r0 = b * T + s * 512 + tt * P
                    nc.sync.dma_start(xn[:], d["x"][r0 : r0 + P, :])
                    for dc in range(DC):
                        pt_ = ptr.tile([P, P], FP32, tag="ptr")
                        nc.tensor.transpose(
                            pt_[:], xn[:, dc * P : (dc + 1) * P], ident[:]
                        )
                        nc.vector.tensor_copy(
                            xt[:, dc, tt * P : (tt + 1) * P], pt_[:]
                        )
                sl = slice(s * 512, (s + 1) * 512)
                # Q and K: project, then RoPE into qt_f/kt_f.
                for name, dst in (("q", qt_f), ("k", kt_f)):
                    pm = pmm.tile([P, 512], FP32, tag="pmm")
                    for dc in range(DC):
                        nc.tensor.matmul(
                            pm[:],
                            w_sb[name][:, dc],
                            xt[:, dc],
                            start=(dc == 0),
                            stop=(dc == DC - 1),
                        )
                    raw = work.tile([P, 512], FP32, tag="raw")
                    nc.vector.tensor_copy(raw[:], pm[:])
                    perm = work.tile([P, 512], FP32, tag="perm")
                    for blk in range(4):
                        p0 = blk * 32
                        src = p0 + 32 if blk % 2 == 0 else p0 - 32
                        nc.sync.dma_start(
                            perm[p0 : p0 + 32, :], raw[src : src + 32, :]
                        )
                    # rope: dst = raw*cos + perm*sin_signed
                    nc.vector.tensor_tensor(raw[:], raw[:], cos_sb[:, sl], ALU.mult)
                    nc.vector.tensor_tensor(perm[:], perm[:], sin_sb[:, sl], ALU.mult)
                    nc.vector.tensor_tensor(dst[:, sl], raw[:], perm[:], ALU.add)
                # V: project then transpose into per-head [tk, 65] tiles.
                pm = pmm.tile([P, 512], FP32, tag="pmm")
                for dc in range(DC):
                    nc.tensor.matmul(
                        pm[:],
                        w_sb["v"][:, dc],
                        xt[:, dc],
                        start=(dc == 0),
                        stop=(dc == DC - 1),
                    )
                vt = work.tile([P, 512], FP32, tag="vt")
                nc.vector.tensor_copy(vt[:], pm[:])
                for tt in range(4):
                    ptv = ptr.tile([P, P], FP32, tag="ptr")
                    nc.tensor.transpose(
                        ptv[:], vt[:, tt * P : (tt + 1) * P], ident[:]
                    )
                    nc.vector.tensor_copy(va[:, s * 4 + tt, 0:64], ptv[:, 0:64])
                    nc.vector.tensor_copy(vb[:, s * 4 + tt, 0:64], ptv[:, 64:128])

            # ---- attention ----
            for s in range(N_STRIPS):
                qsl = slice(s * 512, (s + 1) * 512)
                for h in range(2):
                    ph = 64 * h
                    v_h = va if h == 0 else vb
                    pyt = py.tile([65, 512], FP32, tag="py")
                    jmax = 4 * s + 3
                    for j in range(jmax + 1):
                        pss = ps.tile([P, 512], FP32, tag="ps")
                        nc.tensor.matmul(
                            pss[:],
                            kt_f[ph : ph + 64, j * P : (j + 1) * P],
                            qt_f[ph : ph + 64, qsl],
                            start=True,
                            stop=True,
                        )
                        pt = ptp.tile([P, 512], FP32R, tag="pt")
                        nc.scalar.activation(
                            pt[:], pss[:], AF.Exp, scale=1.0 / math.sqrt(DH)
                        )
                        if j >= 4 * s:
                            # keep where (512s + col) - (128j + row) >= 0
                            nc.gpsimd.affine_select(
                                out=pt[:],
                                in_=pt[:],
                                compare_op=ALU.is_ge,
                                fill=0.0,
                                base=512 * s - 128 * j,
                                channel_multiplier=-1,
                                pattern=[[1, 512]],
                            )
                        nc.tensor.matmul(
                            pyt[:],
                            v_h[:, j, :],
                            pt[:],
                            start=(j == 0),
                            stop=(j == jmax),
                        )
                    r65 = work.tile([65, 512], FP32, tag="r65")
                    nc.vector.reciprocal(r65[64:65, :], pyt[64:65, :])
                    r_dram = dram.tile([1, 512], FP32, tag="r_dram", name="r_dram")
                    nc.sync.dma_start(r_dram[:], r65[64:65, :])
                    rb = work.tile([64, 512], FP32, tag="rb")
                    nc.sync.dma_start(rb[:], r_dram[:].to_broadcast((64, 512)))
                    nc.vector.tensor_tensor(
                        y2t[h][:, b * T + s * 512 : b * T + (s + 1) * 512],
                        pyt[0:64, :],
                        rb[:],
                        ALU.mult,
                    )

    # ---- AllToAll: head-shards -> token-shards ----
    for j in range(TOK // 512):
        jsl = slice(j * 512, (j + 1) * 512)
        nc.sync.dma_start(
            a2a_in[j * P : j * P + 64, :], y2t[0][:, jsl].bitcast(FP32)
        )
        nc.sync.dma_start(
            a2a_in[j * P + 64 : (j + 1) * P, :], y2t[1][:, jsl].bitcast(FP32)
        )
    nc.gpsimd.collective_compute(
        "AllToAll",
        ALU.bypass,
        replica_groups=[list(range(N_CORES))],
        ins=[a2a_in.opt()],
        outs=[a2a_out.opt()],
    )

    # ---- projection of this core's 512 tokens with full W_proj ----
    with (
        tc.tile_pool(name="projc", bufs=1) as projc,
        tc.tile_pool(name="proj", bufs=2) as proj,
    ):
        wp_sb = projc.tile([P, DC, D], FP32R, tag="wp")
        nc.sync.dma_start(
            wp_sb[:], d["wp"][:].rearrange("(o p) j -> p o j", p=P).bitcast(FP32R)
        )
        yt_sb = projc.tile([P, DC, 512], FP32R, tag="yt")
        nc.sync.dma_start(
            yt_sb[:],
            a2a_out[:].rearrange("(o p) t -> p o t", p=P).bitcast(FP32R),
        )
        for tt in range(4):
            for oc in range(2):
                pm = pmm.tile([P, 512], FP32, tag="pmm")
                for dc in range(DC):
                    nc.tensor.matmul(
                        pm[:],
                        yt_sb[:, dc, tt * P : (tt + 1) * P],
                        wp_sb[:, dc, oc * 512 : (oc + 1) * 512],
                        start=(dc == 0),
                        stop=(dc == DC - 1),
                    )
                ob = proj.tile([P, 512], FP32, tag="ob")
                nc.vector.tensor_copy(ob[:], pm[:])
                nc.sync.dma_start(
                    d["out"][tt * P : (tt + 1) * P, oc * 512 : (oc + 1) * 512],
                    ob[:],
                )


def _build_program(reps=1):
    nc = bacc.Bacc(None, target_bir_lowering=False, debug=False)

    d = {
        "x": nc.dram_tensor("x", [TOK, D], FP32, kind="ExternalInput"),
        "wq": nc.dram_tensor("wq", [D, P], FP32, kind="ExternalInput"),
        "wk": nc.dram_tensor("wk", [D, P], FP32, kind="ExternalInput"),
        "wv": nc.dram_tensor("wv", [D, P], FP32, kind="ExternalInput"),
        "wp": nc.dram_tensor("wp", [D, D], FP32, kind="ExternalInput"),
        "cos": nc.dram_tensor("cos", [P, T], FP32, kind="ExternalInput"),
        "sin": nc.dram_tensor("sin", [P, T], FP32, kind="ExternalInput"),
        "ones": nc.dram_tensor("ones", [P, 1], FP32, kind="ExternalInput"),
        "out": nc.dram_tensor("out", [TOK_PER_CORE, D], FP32, kind="ExternalOutput"),
    }

    with tile.TileContext(nc) as tc:
        with (
            tc.tile_pool(name="const", bufs=1) as cpool,
            tc.tile_pool(name="ptr", bufs=2, space="PSUM") as ptr,
            tc.tile_pool(name="pmm", bufs=2, space="PSUM") as pmm,
            tc.tile_pool(name="ps", bufs=2, space="PSUM") as ps,
            tc.tile_pool(name="py", bufs=2, space="PSUM") as py,
            tc.tile_pool(name="dram", bufs=1, space="DRAM") as dram,
        ):
            ident = cpool.tile([P, P], FP32)
            make_identity(nc, ident[:])

            w_sb = {}
            for name in ("q", "k", "v"):
                w_sb[name] = cpool.tile(
                    [P, DC, P], FP32R, tag=f"w{name}", name=f"w{name}"
                )
                nc.sync.dma_start(
                    w_sb[name][:],
                    d[f"w{name}"][:].rearrange("(o p) j -> p o j", p=P).bitcast(FP32R),
                )
            cos_sb = cpool.tile([P, T], FP32)
            sin_sb = cpool.tile([P, T], FP32)
            nc.sync.dma_start(cos_sb[:], d["cos"][:])
            nc.sync.dma_start(sin_sb[:], d["sin"][:])
            ones_sb = cpool.tile([P, 1], FP32)
            nc.sync.dma_start(ones_sb[:], d["ones"][:])

            consts = dict(
                cpool=cpool, dram=dram, ptr=ptr, pmm=pmm, ps=ps, py=py,
                ident=ident, w_sb=w_sb, cos_sb=cos_sb, sin_sb=sin_sb,
                ones_sb=ones_sb,
            )
            for _rep in range(reps):
                _emit_body(nc, tc, d, consts)

    nc.compile()
    return nc


_NC_CACHE = {}


def _get_program(reps=1):
    if reps not in _NC_CACHE:
        _NC_CACHE[reps] = _build_program(reps)
    return _NC_CACHE[reps]


def _host_tables():
    inv_freq = 1.0 / (ROPE_BASE ** (np.arange(0, DH, 2, dtype=np.float32) / DH))
    t = np.arange(T, dtype=np.float32)
    freqs = np.outer(t, inv_freq).astype(np.float32)  # (T, 32)
    cos_t = np.cos(freqs).T                           # (32, T)
    sin_t = np.sin(freqs).T
    cos = np.empty((P, T), np.float32)
    sin = np.empty((P, T), np.float32)
    for blk in range(4):
        cos[blk * 32 : (blk + 1) * 32] = cos_t
        # rotate_half: row p<32 pairs with -q[p+32]; row p>=32 with +q[p-32]
        sgn = -1.0 if blk % 2 == 0 else 1.0
        sin[blk * 32 : (blk + 1) * 32] = sgn * sin_t
    return cos, sin


def make_in_maps(x, W_qkv, W_proj):
    x = np.asarray(x, np.float32).reshape(TOK, D)
    W_qkv = np.asarray(W_qkv, np.float32)
    W_proj = np.asarray(W_proj, np.float32)
    cos, sin = _host_tables()
    ones = np.ones((P, 1), np.float32)

    in_maps = []
    for c in range(N_CORES):
        j0 = c * P
        in_maps.append(
            {
                "x": x,
                "wq": np.ascontiguousarray(W_qkv[:, j0 : j0 + P]),
                "wk": np.ascontiguousarray(W_qkv[:, D + j0 : D + j0 + P]),
                "wv": np.ascontiguousarray(W_qkv[:, 2 * D + j0 : 2 * D + j0 + P]),
                "wp": W_proj,
                "cos": cos,
                "sin": sin,
                "ones": ones,
            }
        )
    return in_maps


def kernel(x, W_qkv, W_proj):
    in_maps = make_in_maps(x, W_qkv, W_proj)
    nc = _get_program()
    res = run_bass_kernel_spmd(nc, in_maps, list(range(N_CORES)))
    out = np.concatenate([res.results[c]["out"] for c in range(N_CORES)], axis=0)
    return out.reshape(B, T, D)


# revision 11
# speedup vs baseline: 100.3598x; 1.0365x over previous
"""Causal self-attention with RoPE on 8 Trainium2 NeuronCores.

Sharding: tensor-parallel over heads (2 heads/core) for QKV+attention,
then an AllToAll redistributes y^T from head-shards to token-shards and
each core projects its 512-token slice with the full W_proj.

All matmuls run in float32r (TF32) with fp32 PSUM accumulation.
"""

import math

import numpy as np

import concourse.bass as bass
import concourse.mybir as mybir
import concourse.tile as tile
from concourse import bacc
from concourse.bass_utils import run_bass_kernel_spmd
from concourse.masks import make_identity

# Problem shape (hardcoded per contest rules).
B, T, D = 2, 2048, 1024
H, DH = 16, 64
ROPE_BASE = 10000.0
N_CORES = 8
HEADS_PER_CORE = H // N_CORES          # 2
P = 128
N_STRIPS = T // 512                    # 4 strips of 512 tokens per batch
TOK = B * T                            # 4096 flat tokens
TOK_PER_CORE = TOK // N_CORES          # 512
DC = D // P                            # 8 contraction chunks

FP32 = mybir.dt.float32
FP32R = mybir.dt.float32r
AF = mybir.ActivationFunctionType
ALU = mybir.AluOpType


def _emit_body(nc, tc, d, consts):
    """One full forward pass; emitted `reps` times for slope timing."""
    dram = consts["dram"]
    ptr, ps, pmm, py = consts["ptr"], consts["ps"], consts["pmm"], consts["py"]
    ident, w_sb = consts["ident"], consts["w_sb"]
    cos_sb, sin_sb, ones_sb = consts["cos_sb"], consts["sin_sb"], consts["ones_sb"]

    with (
        tc.tile_pool(name="ybuf", bufs=1) as ybuf,
        tc.tile_pool(name="work", bufs=2) as work,
        tc.tile_pool(name="ptp", bufs=3) as ptp,
        tc.tile_pool(name="projp", bufs=2) as projp,
        tc.tile_pool(name="wpp", bufs=1) as wpp,
    ):
        for b in range(B):
            # y^T accumulators for this batch (head A / head B rows in separate
            # tiles so every DVE write stays lane-aligned).
            y2t = [ybuf.tile([64, T], FP32R, tag=f"y2t{h}", name=f"y2t{h}")
                   for h in range(2)]
            qt_f = work.tile([P, T], FP32R, tag="qt_f")
            kt_f = work.tile([P, T], FP32R, tag="kt_f")
            va = work.tile([P, T // P, 65], FP32R, tag="va")
            vb = work.tile([P, T // P, 65], FP32R, tag="vb")
            nc.gpsimd.tensor_copy(
                va[:, :, 64], ones_sb[:, 0:1].to_broadcast((P, T // P))
            )
            nc.gpsimd.tensor_copy(
                vb[:, :, 64], ones_sb[:, 0:1].to_broadcast((P, T // P))
            )

            # ---- QKV projection with fused per-d-chunk x transpose ----
            for s in range(N_STRIPS):
                xns = []
                for tt in range(4):
                    xn = work.tile([P, D], FP32, tag=f"xn{tt}", name="xn")
                    r0 = b * T + s * 512 + tt * P
                    nc.sync.dma_start(xn[:], d["x"][r0 : r0 + P, :])
                    xns.append(xn)
                pm = pmm.tile([P, 3, 512], FP32, tag="pmm")
                for dc in range(DC):
                    ptile = ptr.tile([P, 4, P], FP32, tag="ptr")
                    for tt in range(4):
                        nc.tensor.transpose(
                            ptile[:, tt], xns[tt][:, dc * P : (dc + 1) * P], ident[:]
                        )
                    xtc = work.tile([P, 512], FP32R, tag="xtc")
                    nc.vector.tensor_copy(xtc[:], ptile[:])
                    for i in range(3):
                        nc.tensor.matmul(
                            pm[:, i],
                            w_sb["qkv"[i]][:, dc],
                            xtc[:],
                            start=(dc == 0),
                            stop=(dc == DC - 1),
                        )
                sl = slice(s * 512, (s + 1) * 512)
                # Q and K: RoPE into qt_f/kt_f.
                for i, dst in ((0, qt_f), (1, kt_f)):
                    raw = work.tile([P, 512], FP32, tag="raw")
                    nc.vector.tensor_copy(raw[:], pm[:, i])
                    perm = work.tile([P, 512], FP32, tag="perm")
                    for blk in range(4):
                        p0 = blk * 32
                        src = p0 + 32 if blk % 2 == 0 else p0 - 32
                        nc.sync.dma_start(
                            perm[p0 : p0 + 32, :], raw[src : src + 32, :]
                        )
                    # rope: dst = raw*cos + perm*sin_signed
                    nc.gpsimd.tensor_tensor(raw[:], raw[:], cos_sb[:, sl], ALU.mult)
                    nc.gpsimd.tensor_tensor(perm[:], perm[:], sin_sb[:, sl], ALU.mult)
                    nc.vector.tensor_tensor(dst[:, sl], raw[:], perm[:], ALU.add)
                # V: transpose into per-head [tk, 65] tiles.
                vt = work.tile([P, 512], FP32, tag="vt")
                nc.vector.tensor_copy(vt[:], pm[:, 2])
                ptile2 = ptr.tile([P, 4, P], FP32, tag="ptr")
                for tt in range(4):
                    nc.tensor.transpose(
                        ptile2[:, tt], vt[:, tt * P : (tt + 1) * P], ident[:]
                    )
                nc.vector.tensor_copy(
                    va[:, s * 4 : s * 4 + 4, 0:64], ptile2[:, :, 0:64]
                )
                nc.vector.tensor_copy(
                    vb[:, s * 4 : s * 4 + 4, 0:64], ptile2[:, :, 64:128]
                )

            # ---- attention ----
            for s in range(N_STRIPS):
                qsl = slice(s * 512, (s + 1) * 512)
                for h in range(2):
                    ph = 64 * h
                    v_h = va if h == 0 else vb
                    pyt = py.tile([65, 512], FP32, tag="py")
                    jmax = 4 * s + 3
                    for j in range(jmax + 1):
                        pss = ps.tile([P, 512], FP32, tag="ps")
                        nc.tensor.matmul(
                            pss[:],
                            kt_f[ph : ph + 64, j * P : (j + 1) * P],
                            qt_f[ph : ph + 64, qsl],
                            start=True,
                            stop=True,
                        )
                        pt = ptp.tile([P, 512], FP32R, tag="pt")
                        nc.scalar.activation(
                            pt[:], pss[:], AF.Exp, scale=1.0 / math.sqrt(DH)
                        )
                        if j >= 4 * s:
                            # keep where (512s + col) - (128j + row) >= 0
                            nc.gpsimd.affine_select(
                                out=pt[:],
                                in_=pt[:],
                                compare_op=ALU.is_ge,
                                fill=0.0,
                                base=512 * s - 128 * j,
                                channel_multiplier=-1,
                                pattern=[[1, 512]],
                            )
                        nc.tensor.matmul(
                            pyt[:],
                            v_h[:, j, :],
                            pt[:],
                            start=(j == 0),
                            stop=(j == jmax),
                        )
                    r65 = work.tile([65, 512], FP32, tag="r65")
                    nc.vector.reciprocal(r65[64:65, :], pyt[64:65, :])
                    r_dram = dram.tile([1, 512], FP32, tag="r_dram", name="r_dram")
                    nc.sync.dma_start(r_dram[:], r65[64:65, :])
                    rb = work.tile([64, 512], FP32, tag="rb")
                    nc.sync.dma_start(rb[:], r_dram[:].to_broadcast((64, 512)))
                    nc.vector.tensor_tensor(
                        y2t[h][:, qsl], pyt[0:64, :], rb[:], ALU.mult
                    )

            # ---- per-batch AllToAll: head-shards -> 256-token shards ----
            a2a_in = dram.tile([N_CORES * P, 256], FP32, tag="a2a_in",
                               name="a2a_in")
            a2a_out = dram.tile([N_CORES * P, 256], FP32, tag="a2a_out",
                                name="a2a_out")
            for j in range(N_CORES):
                jsl = slice(j * 256, (j + 1) * 256)
                nc.sync.dma_start(
                    a2a_in[j * P : j * P + 64, :], y2t[0][:, jsl].bitcast(FP32)
                )
                nc.sync.dma_start(
                    a2a_in[j * P + 64 : (j + 1) * P, :], y2t[1][:, jsl].bitcast(FP32)
                )
            nc.gpsimd.collective_compute(
                "AllToAll",
                ALU.bypass,
                replica_groups=[list(range(N_CORES))],
                ins=[a2a_in.opt()],
                outs=[a2a_out.opt()],
            )

            # ---- projection of this core's 256 tokens (this batch) ----
            yt_sb = projp.tile([P, DC, 256], FP32R, tag="yt")
            nc.sync.dma_start(
                yt_sb[:],
                a2a_out[:].rearrange("(o p) t -> p o t", p=P).bitcast(FP32R),
            )
            for oc in range(2):
                wpc = wpp.tile([P, DC, 512], FP32R, tag="wpc")
                nc.sync.dma_start(
                    wpc[:],
                    d["wp"][:, oc * 512 : (oc + 1) * 512]
                    .rearrange("(o p) j -> p o j", p=P)
                    .bitcast(FP32R),
                )
                for tt in range(2):
                    pmo = py.tile([P, 512], FP32, tag="py")
                    for dc in range(DC):
                        nc.tensor.matmul(
                            pmo[:],
                            yt_sb[:, dc, tt * P : (tt + 1) * P],
                            wpc[:, dc],
                            start=(dc == 0),
                            stop=(dc == DC - 1),
                        )
                    ob = projp.tile([P, 512], FP32, tag="ob")
                    nc.vector.tensor_copy(ob[:], pmo[:])
                    nc.sync.dma_start(
                        d["out"][
                            b * 256 + tt * P : b * 256 + (tt + 1) * P,
                            oc * 512 : (oc + 1) * 512,
                        ],
                        ob[:],
                    )


def _build_program(reps=1):
    nc = bacc.Bacc(None, target_bir_lowering=False, debug=False)

    d = {
        "x": nc.dram_tensor("x", [TOK, D], FP32, kind="ExternalInput"),
        "wq": nc.dram_tensor("wq", [D, P], FP32, kind="ExternalInput"),
        "wk": nc.dram_tensor("wk", [D, P], FP32, kind="ExternalInput"),
        "wv": nc.dram_tensor("wv", [D, P], FP32, kind="ExternalInput"),
        "wp": nc.dram_tensor("wp", [D, D], FP32, kind="ExternalInput"),
        "cos": nc.dram_tensor("cos", [P, T], FP32, kind="ExternalInput"),
        "sin": nc.dram_tensor("sin", [P, T], FP32, kind="ExternalInput"),
        "ones": nc.dram_tensor("ones", [P, 1], FP32, kind="ExternalInput"),
        "out": nc.dram_tensor("out", [TOK_PER_CORE, D], FP32, kind="ExternalOutput"),
    }

    with tile.TileContext(nc) as tc:
        with (
            tc.tile_pool(name="const", bufs=1) as cpool,
            tc.tile_pool(name="ptr", bufs=1, space="PSUM") as ptr,
            tc.tile_pool(name="ps", bufs=2, space="PSUM") as ps,
            tc.tile_pool(name="pmm", bufs=1, space="PSUM") as pmm,
            tc.tile_pool(name="py", bufs=2, space="PSUM") as py,
            tc.tile_pool(name="dram", bufs=1, space="DRAM") as dram,
        ):
            ident = cpool.tile([P, P], FP32)
            make_identity(nc, ident[:])

            w_sb = {}
            for name in ("q", "k", "v"):
                w_sb[name] = cpool.tile(
                    [P, DC, P], FP32R, tag=f"w{name}", name=f"w{name}"
                )
                nc.sync.dma_start(
                    w_sb[name][:],
                    d[f"w{name}"][:].rearrange("(o p) j -> p o j", p=P).bitcast(FP32R),
                )
            cos_sb = cpool.tile([P, T], FP32)
            sin_sb = cpool.tile([P, T], FP32)
            nc.sync.dma_start(cos_sb[:], d["cos"][:])
            nc.sync.dma_start(sin_sb[:], d["sin"][:])
            ones_sb = cpool.tile([P, 1], FP32)
            nc.sync.dma_start(ones_sb[:], d["ones"][:])

            consts = dict(
                cpool=cpool, dram=dram, ptr=ptr, ps=ps, pmm=pmm, py=py,
                ident=ident, w_sb=w_sb, cos_sb=cos_sb, sin_sb=sin_sb,
                ones_sb=ones_sb,
            )
            for _rep in range(reps):
                _emit_body(nc, tc, d, consts)

    nc.compile()
    return nc


_NC_CACHE = {}


def _get_program(reps=1):
    if reps not in _NC_CACHE:
        _NC_CACHE[reps] = _build_program(reps)
    return _NC_CACHE[reps]


def _host_tables():
    inv_freq = 1.0 / (ROPE_BASE ** (np.arange(0, DH, 2, dtype=np.float32) / DH))
    t = np.arange(T, dtype=np.float32)
    freqs = np.outer(t, inv_freq).astype(np.float32)  # (T, 32)
    cos_t = np.cos(freqs).T                           # (32, T)
    sin_t = np.sin(freqs).T
    cos = np.empty((P, T), np.float32)
    sin = np.empty((P, T), np.float32)
    for blk in range(4):
        cos[blk * 32 : (blk + 1) * 32] = cos_t
        # rotate_half: row p<32 pairs with -q[p+32]; row p>=32 with +q[p-32]
        sgn = -1.0 if blk % 2 == 0 else 1.0
        sin[blk * 32 : (blk + 1) * 32] = sgn * sin_t
    return cos, sin


def make_in_maps(x, W_qkv, W_proj):
    x = np.asarray(x, np.float32).reshape(TOK, D)
    W_qkv = np.asarray(W_qkv, np.float32)
    W_proj = np.asarray(W_proj, np.float32)
    cos, sin = _host_tables()
    ones = np.ones((P, 1), np.float32)

    in_maps = []
    for c in range(N_CORES):
        j0 = c * P
        in_maps.append(
            {
                "x": x,
                "wq": np.ascontiguousarray(W_qkv[:, j0 : j0 + P]),
                "wk": np.ascontiguousarray(W_qkv[:, D + j0 : D + j0 + P]),
                "wv": np.ascontiguousarray(W_qkv[:, 2 * D + j0 : 2 * D + j0 + P]),
                "wp": W_proj,
                "cos": cos,
                "sin": sin,
                "ones": ones,
            }
        )
    return in_maps


def kernel(x, W_qkv, W_proj):
    in_maps = make_in_maps(x, W_qkv, W_proj)
    nc = _get_program()
    res = run_bass_kernel_spmd(nc, in_maps, list(range(N_CORES)))
    return assemble([res.results[c]["out"] for c in range(N_CORES)])


def assemble(outs):
    full = np.empty((B, T, D), np.float32)
    for c in range(N_CORES):
        o = outs[c]
        for b in range(B):
            full[b, 256 * c : 256 * (c + 1)] = o[b * 256 : (b + 1) * 256]
    return full
